# revision 1
# baseline (speedup 1.0000x reference)
"""Trainium2 Bass kernel for the DeepBayesianFilterBlockDiag loss.

Strategy (8-core SPMD, observation-axis sharded):
  - The 152064-dim observation axis is split into 8 shards of 19008 columns.
    Each core gets its shard of target [256,19008], W_dec||b_dec [65,19008],
    log_R [19008], plus the full (tiny) per-(b,t,z) tensors.
  - Per core:
      * compute Xe = [mu_f + chol(sigma_f) @ eps, 1] (tiny 2x2 algebra),
        transposed+negated into a [65,256] lhsT (bf16 copy for the GEMM).
      * main loop over 1024-column groups: PE "injects" the target into PSUM
        via an identity matmul, then accumulates -Xe @ W' (bf16 weights) on
        top, leaving d = t - rec in PSUM with zero DVE work.  ACT squares d
        into SBUF (bf16); PE reduces rows per 128-column chunk with a
        transposed matmul (lhsT = squares, rhs = ones) so the column sums
        land on partitions.
      * epilogue: weight the column sums by exp(-2 log_R) (transposed onto
        partitions via PE), reduce log_R and the KL partials, emit a [5]
        vector (sse_mg0, sse_mg1, sum_logR_main, sum_logR_rem, kl_raw).
  - Host combines the 8 partial vectors into the final scalar loss.
"""

import math

import numpy as np

import concourse.bass as bass
import concourse.mybir as mybir
import concourse.tile as tile
from concourse.bass_utils import run_bass_kernel_spmd
from concourse.masks import make_identity

F32 = mybir.dt.float32
BF16 = mybir.dt.bfloat16
AF = mybir.ActivationFunctionType
OP = mybir.AluOpType

B, T, Z, DIM = 4, 64, 32, 2
ROWS = B * T          # 256
LAT = Z * DIM         # 64
LATP = LAT + 1        # 65 (ones row folds in b_dec)
D_OBS = 152064
NCORES = 8
DC = D_OBS // NCORES  # 19008 columns per core
CH = 512              # psum-bank column chunk
GRP = 1024            # ACT / psum group (2 chunks)
N_FULL = DC // CH     # 37 full chunks
REM = DC - N_FULL * CH  # 64

CCH = 128             # colsum chunk (transposed-reduce matmul width)
MAX_DRAIN_WAITS = 1
USE_INJECT = True
ABLATE = set()  # perf-debug: subset of {"phase1","inject","mains","square","colsum","dma_t","dma_wb"}


def _layout(dc):
    groups = []
    off = 0
    while off < dc:
        g = []
        goff = off
        for _ in range(GRP // CH):
            w = min(CH, dc - off)
            if w <= 0:
                break
            g.append((off - goff, w))
            off += w
        groups.append((goff, g))
    n_full = dc // CH
    rem = dc - n_full * CH
    ncc = (dc + CCH - 1) // CCH
    return groups, n_full, rem, ncc


def _split_multi_waits(nc, max_waits=1):
    """walrus' per-instruction sync encoding only fits one wait; move extra
    waits emitted by Tile onto NOPs inserted just before the instruction on
    the same engine (same semantics: engine blocks on all of them in order).
    """
    k = 0
    for f in nc.m.functions:
        for blk in f.blocks:
            il = blk.instructions
            i = 0
            while i < len(il):
                inst = il[i]
                si = inst.sync_info
                if si is not None and len(si.on_wait) > max_waits:
                    waits = list(si.on_wait)
                    inst.sync_info = mybir.SyncInfo(
                        on_wait=waits[-max_waits:], on_update=list(si.on_update)
                    )
                    extra = waits[:-max_waits]
                    for j in range(0, len(extra), max_waits):
                        nop = mybir.InstEventSemaphore(
                            name=f"{inst.name}-w{k}",
                            engine=inst.engine,
                            sync_info=mybir.SyncInfo(
                                on_wait=extra[j : j + max_waits], on_update=[]
                            ),
                        )
                        k += 1
                        il.insert(i, nop)
                        i += 1
                i += 1


def _comp4(t, mg, idx):
    # [128, 2, 128] tile -> [128, 32] view of 2x2-block component idx
    return t[:, mg, :].rearrange("p (z k) -> p z k", k=4)[:, :, idx]


def _comp2(t, mg, idx):
    return t[:, mg, :].rearrange("p (z k) -> p z k", k=2)[:, :, idx]


def build_nc(reps: int = 1, dc: int = DC, split_waits: bool = True, dup: int = 1):
    nc = bass.Bass("TRN2")
    tgt = nc.dram_tensor("tgt", [ROWS, dc], F32, kind="ExternalInput")
    wb = nc.dram_tensor("wb", [LATP, dc], F32, kind="ExternalInput")
    lr = nc.dram_tensor("log_r", [dc], F32, kind="ExternalInput")
    muf = nc.dram_tensor("mu_f", [ROWS, LAT], F32, kind="ExternalInput")
    sgf = nc.dram_tensor("sig_f", [ROWS, 4 * Z], F32, kind="ExternalInput")
    mup = nc.dram_tensor("mu_p", [ROWS, LAT], F32, kind="ExternalInput")
    sgp = nc.dram_tensor("sig_p", [ROWS, 4 * Z], F32, kind="ExternalInput")
    eps = nc.dram_tensor("eps", [ROWS, LAT], F32, kind="ExternalInput")
    out = nc.dram_tensor("out", [5], F32, kind="ExternalOutput")

    with tile.TileContext(nc) as tc:
        with (
            tc.tile_pool(name="big", bufs=1) as big,
            tc.tile_pool(name="tp", bufs=8) as tpool,
            tc.tile_pool(name="sp", bufs=3) as spool,
            tc.tile_pool(name="small", bufs=1) as small,
            tc.tile_pool(name="dps", bufs=3, space="PSUM") as dpsum,
            tc.tile_pool(name="acc", bufs=1, space="PSUM") as accpsum,
            tc.tile_pool(name="smallps", bufs=1, space="PSUM") as smallps,
        ):
            if reps == 1:
                _body(nc, tc, big, tpool, spool, small, dpsum, accpsum, smallps,
                      tgt, wb, lr, muf, sgf, mup, sgp, eps, out, dc)
            else:
                with tc.For_i(0, reps, 1):
                    for _ in range(dup):
                        _body(nc, tc, big, tpool, spool, small, dpsum, accpsum,
                              smallps, tgt, wb, lr, muf, sgf, mup, sgp, eps,
                              out, dc)
    if split_waits:
        # needed for the walrus/HW path; CoreSim wants the raw form
        _split_multi_waits(nc)
    return nc


def _body(nc, tc, big, tpool, spool, small, dpsum, accpsum, smallps,
          tgt, wb, lr, muf, sgf, mup, sgp, eps, out, dc=DC):
    GROUPS, N_FULL, REM, NCC = _layout(dc)
    DCL = dc
    ident = small.tile([128, 128], F32)
    make_identity(nc, ident)
    ones = small.tile([128, 1], F32)
    nc.vector.memset(ones, 1.0)
    ones_bf = small.tile([128, 1], BF16)
    nc.vector.memset(ones_bf, 1.0)

    # ---- small inputs ----
    sigf_s = small.tile([128, 2, 4 * Z], F32)
    sigp_s = small.tile([128, 2, 4 * Z], F32)
    muf_s = small.tile([128, 2, LAT], F32)
    mup_s = small.tile([128, 2, LAT], F32)
    eps_s = small.tile([128, 2, LAT], F32)
    for mg in range(2):
        rs = slice(mg * 128, (mg + 1) * 128)
        nc.sync.dma_start(out=sigf_s[:, mg, :], in_=sgf[rs, :])
        nc.sync.dma_start(out=sigp_s[:, mg, :], in_=sgp[rs, :])
        nc.sync.dma_start(out=muf_s[:, mg, :], in_=muf[rs, :])
        nc.sync.dma_start(out=mup_s[:, mg, :], in_=mup[rs, :])
        nc.sync.dma_start(out=eps_s[:, mg, :], in_=eps[rs, :])

    lr37 = small.tile([N_FULL, CH], F32)
    lrrem = small.tile([1, REM], F32)
    nc.sync.dma_start(
        out=lr37, in_=lr[0 : N_FULL * CH].rearrange("(p f) -> p f", f=CH)
    )
    nc.sync.dma_start(
        out=lrrem, in_=lr[N_FULL * CH : DCL].rearrange("(p f) -> p f", f=REM)
    )

    # ---- phase 1: Xe (cholesky sample) + KL, per 128-row group ----
    lhsT = small.tile([LATP, 256], F32)
    nc.vector.memset(lhsT[LAT:LATP, :], -1.0)
    kl2 = small.tile([128, 2], F32)

    if "phase1" in ABLATE:
        nc.vector.memset(lhsT, 0.01)
        nc.vector.memset(kl2, 0.0)
    for mg in range(2 if "phase1" not in ABLATE else 0):
        af = _comp4(sigf_s, mg, 0)
        bf = _comp4(sigf_s, mg, 1)
        cf = _comp4(sigf_s, mg, 2)
        df = _comp4(sigf_s, mg, 3)
        aq = _comp4(sigp_s, mg, 0)
        bq = _comp4(sigp_s, mg, 1)
        cq = _comp4(sigp_s, mg, 2)
        dq = _comp4(sigp_s, mg, 3)

        # cholesky: l11 = sqrt(a); l21 = c/l11; l22 = sqrt(d - l21^2)
        l11 = small.tile([128, Z], F32)
        nc.scalar.sqrt(l11, af)
        r11 = small.tile([128, Z], F32)
        nc.vector.reciprocal(r11, l11)
        l21 = small.tile([128, Z], F32)
        nc.vector.tensor_mul(l21, cf, r11)
        tmp0 = small.tile([128, Z], F32)
        nc.vector.tensor_mul(tmp0, l21, l21)
        nc.vector.tensor_sub(tmp0, df, tmp0)
        l22 = small.tile([128, Z], F32)
        nc.scalar.sqrt(l22, tmp0)

        e1 = _comp2(eps_s, mg, 0)
        e2 = _comp2(eps_s, mg, 1)
        m1 = _comp2(muf_s, mg, 0)
        m2 = _comp2(muf_s, mg, 1)

        xew = small.tile([128, LAT], F32)
        x1v = xew.rearrange("p (z k) -> p z k", k=2)[:, :, 0]
        x2v = xew.rearrange("p (z k) -> p z k", k=2)[:, :, 1]
        tA = small.tile([128, Z], F32)
        nc.vector.tensor_mul(tA, l11, e1)
        nc.vector.tensor_add(x1v, tA, m1)
        tB = small.tile([128, Z], F32)
        nc.vector.tensor_mul(tB, l21, e1)
        tC = small.tile([128, Z], F32)
        nc.vector.tensor_mul(tC, l22, e2)
        nc.vector.tensor_add(tB, tB, tC)
        nc.vector.tensor_add(x2v, tB, m2)

        tps = smallps.tile([LAT, 128], F32, tag="sps")
        nc.tensor.transpose(tps, xew, ident)
        nc.scalar.mul(lhsT[0:LAT, mg * 128 : (mg + 1) * 128], tps, -1.0)

        # KL pieces
        detq = small.tile([128, Z], F32)
        tD = small.tile([128, Z], F32)
        nc.vector.tensor_mul(detq, aq, dq)
        nc.vector.tensor_mul(tD, bq, cq)
        nc.vector.tensor_sub(detq, detq, tD)
        detp = small.tile([128, Z], F32)
        nc.vector.tensor_mul(detp, af, df)
        nc.vector.tensor_mul(tD, bf, cf)
        nc.vector.tensor_sub(detp, detp, tD)
        rdq = small.tile([128, Z], F32)
        nc.vector.reciprocal(rdq, detq)

        # trace numerator: dq*af - bq*bf - cq*cf + aq*df
        tn = small.tile([128, Z], F32)
        nc.vector.tensor_mul(tn, dq, af)
        nc.vector.tensor_mul(tD, aq, df)
        nc.vector.tensor_add(tn, tn, tD)
        nc.vector.tensor_mul(tD, bq, bf)
        nc.vector.tensor_sub(tn, tn, tD)
        nc.vector.tensor_mul(tD, cq, cf)
        nc.vector.tensor_sub(tn, tn, tD)

        # quad numerator: dq*d1^2 - (bq+cq)*d1*d2 + aq*d2^2
        p1 = _comp2(mup_s, mg, 0)
        p2 = _comp2(mup_s, mg, 1)
        d1 = small.tile([128, Z], F32)
        nc.vector.tensor_sub(d1, p1, m1)
        d2 = small.tile([128, Z], F32)
        nc.vector.tensor_sub(d2, p2, m2)
        qn = small.tile([128, Z], F32)
        nc.vector.tensor_mul(tD, d1, d1)
        nc.vector.tensor_mul(qn, dq, tD)
        nc.vector.tensor_mul(tD, d2, d2)
        nc.vector.tensor_mul(tD, aq, tD)
        nc.vector.tensor_add(qn, qn, tD)
        nc.vector.tensor_mul(tD, d1, d2)
        tE = small.tile([128, Z], F32)
        nc.vector.tensor_add(tE, bq, cq)
        nc.vector.tensor_mul(tD, tD, tE)
        nc.vector.tensor_sub(qn, qn, tD)

        klv = small.tile([128, Z], F32)
        nc.vector.tensor_add(klv, tn, qn)
        nc.vector.tensor_mul(klv, klv, rdq)
        # + ln(detq) - ln(detp)
        nc.scalar.activation(tD, detq, AF.Ln)
        nc.vector.tensor_add(klv, klv, tD)
        nc.scalar.activation(tD, detp, AF.Ln)
        nc.vector.tensor_sub(klv, klv, tD)
        nc.vector.reduce_sum(out=kl2[:, mg : mg + 1], in_=klv, axis=mybir.AxisListType.X)

    # w = exp(-2 log_R) (same ACT table set as Ln)
    w37 = small.tile([N_FULL, CH], F32)
    nc.scalar.activation(w37, lr37, AF.Exp, scale=-2.0)
    wrem = small.tile([1, REM], F32)
    nc.scalar.activation(wrem, lrrem, AF.Exp, scale=-2.0)

    # transpose w into [128, NCC]: wfull[p, cc] = w[cc*128 + p]
    wfull = small.tile([128, (N_FULL + 1) * (CH // CCH)], F32)  # [128, 152]
    nc.vector.memset(wfull, 0.0)
    wview = wfull.rearrange("p (r j) -> p r j", j=CH // CCH)  # [128, 38, 4]
    for j in range(CH // CCH):
        wtp = smallps.tile([128, N_FULL], F32, tag="sps")
        nc.tensor.transpose(wtp, w37[:, j * CCH : (j + 1) * CCH], ident[0:N_FULL, 0:N_FULL])
        nc.scalar.copy(wview[:, 0:N_FULL, j], wtp)
    wtr = smallps.tile([REM, 1], F32, tag="sps")
    nc.tensor.transpose(wtr, wrem, ident[0:1, 0:1])
    nc.scalar.copy(wfull[0:REM, NCC - 1 : NCC], wtr)

    # bf16 copies for the PE weight-heavy operands (fp32 LDWEIGHTS is 4x slow)
    lhsT_bf = small.tile([LATP, 256], BF16)
    nc.vector.tensor_copy(lhsT_bf, lhsT)

    # ---- W' (with b_dec row) resident in SBUF ----
    # loaded upfront on the second HWDGE ring (ScalarE) so it drains in
    # parallel with the target stream on the sync ring
    wb_s = big.tile([LATP, DCL], F32)
    wb_bf = big.tile([LATP, DCL], BF16)
    if "dma_wb" not in ABLATE:
        for woff in range(0, DCL, 2048):
            ww = min(2048, DCL - woff)
            nc.scalar.dma_start(
                out=wb_s[:, woff : woff + ww], in_=wb[:, woff : woff + ww]
            )
            nc.vector.tensor_copy(
                wb_bf[:, woff : woff + ww], wb_s[:, woff : woff + ww]
            )

    # colsum bank: column-sums of squares land on partitions.
    # mg0 -> free slots [0, NCC), mg1 -> [256, 256+NCC)
    colsum = accpsum.tile([128, 512], F32)
    nc.vector.memset(colsum, 0.0)

    # ---- phase 2: main loop ----
    first_mg = True
    SG = 2048
    sgs = []
    off = 0
    while off < DCL:
        w_ = min(SG, DCL - off)
        sgs.append((off, w_))
        off += w_
    for mg in range(2):
        rs = slice(mg * 128, (mg + 1) * 128)
        lhsT_mg = lhsT[:, mg * 128 : (mg + 1) * 128]
      # doubled target DMAs (amortize per-DMA completion latency)
        for soff, sw in sgs:
            t_s = tpool.tile([128, SG], F32)
            if "dma_t" not in ABLATE:
                nc.sync.dma_start(out=t_s[:, 0:sw], in_=tgt[rs, soff : soff + sw])
            for ioff in range(0, sw, GRP):
                gw = min(GRP, sw - ioff)
                goff = soff + ioff
                chunks = [(c, min(CH, gw - c)) for c in range(0, gw, CH)]
                t_v = t_s[:, ioff : ioff + gw]
                dps = dpsum.tile([128, GRP], F32)
                if "inject" in ABLATE or "mains" in ABLATE:
                    if "inject" not in ABLATE:
                        for coff, cw in chunks:
                            nc.tensor.matmul(
                                dps[:, coff : coff + cw], lhsT=ident,
                                rhs=t_v[:, coff : coff + cw], start=True, stop=True)
                    elif "mains" not in ABLATE:
                        for coff, cw in chunks:
                            nc.tensor.matmul(
                                dps[:, coff : coff + cw], lhsT=lhsT_mg,
                                rhs=wb_s[:, goff + coff : goff + coff + cw],
                                start=True, stop=True)
                    else:
                        nc.vector.memset(dps[:, 0:gw], 0.0)
                elif USE_INJECT:
                    for coff, cw in chunks:
                        nc.tensor.matmul(
                            dps[:, coff : coff + cw],
                            lhsT=ident,
                            rhs=t_v[:, coff : coff + cw],
                            start=True,
                            stop=False,
                        )
                    for coff, cw in chunks:
                        nc.tensor.matmul(
                            dps[:, coff : coff + cw],
                            lhsT=lhsT_bf[:, mg * 128 : (mg + 1) * 128],
                            rhs=wb_bf[:, goff + coff : goff + coff + cw],
                            start=False,
                            stop=True,
                        )
                else:
                    for coff, cw in chunks:
                        nc.tensor.matmul(
                            dps[:, coff : coff + cw],
                            lhsT=lhsT_mg,
                            rhs=wb_s[:, goff + coff : goff + coff + cw],
                            start=True,
                            stop=True,
                        )
                    # d = t + (-Xe @ W'), in place in PSUM
                    nc.vector.tensor_add(dps[:, 0:gw], t_v[:, 0:gw], dps[:, 0:gw])
                s_s = spool.tile([128, GRP], BF16)
                if "square" not in ABLATE:
                    nc.scalar.square(s_s[:, 0:gw], dps[:, 0:gw])
                elif first_mg and goff == 0:
                    nc.vector.memset(s_s, 0.0)
                # transposed column reduce: out[c, 0] = sum_rows s[row, c]
                for j in range((gw + CCH - 1) // CCH if "colsum" not in ABLATE else 0):
                    cw = min(CCH, gw - j * CCH)
                    slot = mg * 256 + goff // CCH + j
                    nc.tensor.matmul(
                        colsum[0:cw, slot : slot + 1],
                        lhsT=s_s[:, j * CCH : j * CCH + cw],
                        rhs=ones_bf,
                        start=True,
                        stop=True,
                    )
        first_mg = False

    # ---- phase 3: epilogue ----
    # combo columns: 0 = sse(mg0), 1 = sse(mg1), 2 = sum(logR) main,
    #                3 = sum(logR) remainder, 4 = kl_raw
    combo = small.tile([128, 5], F32)
    nc.vector.memset(combo, 0.0)

    prod = small.tile([128, NCC], F32)
    for mg in range(2):
        nc.vector.tensor_mul(prod, colsum[:, mg * 256 : mg * 256 + NCC], wfull[:, 0:NCC])
        nc.vector.reduce_sum(
            out=combo[:, mg : mg + 1], in_=prod, axis=mybir.AxisListType.X
        )

    nc.vector.reduce_sum(out=combo[0:N_FULL, 2:3], in_=lr37, axis=mybir.AxisListType.X)
    nc.vector.reduce_sum(out=combo[0:1, 3:4], in_=lrrem, axis=mybir.AxisListType.X)
    nc.vector.tensor_add(combo[:, 4:5], kl2[:, 0:1], kl2[:, 1:2])

    fps = smallps.tile([5, 1], F32, tag="sps")
    nc.tensor.matmul(fps, lhsT=combo, rhs=ones, start=True, stop=True)
    res = small.tile([5, 1], F32)
    nc.scalar.copy(res, fps)
    nc.sync.dma_start(out=out[:].rearrange("(p f) -> p f", f=1), in_=res)


_CACHED_NC = {}


def _get_nc(reps: int = 1):
    key = (reps, frozenset(ABLATE))
    if key not in _CACHED_NC:
        _CACHED_NC[key] = build_nc(reps)
    return _CACHED_NC[key]


def make_in_maps(mu_filtered, sigma_filtered, mu_pred, sigma_pred, target,
                 W_dec, b_dec, log_R, eps):
    tgt = np.asarray(target, dtype=np.float32).reshape(ROWS, D_OBS)
    wbf = np.concatenate(
        [np.asarray(W_dec, dtype=np.float32),
         np.asarray(b_dec, dtype=np.float32)[None, :]], axis=0
    )
    lr = np.asarray(log_R, dtype=np.float32)
    smalls = {
        "mu_f": np.ascontiguousarray(
            np.asarray(mu_filtered, dtype=np.float32).reshape(ROWS, LAT)),
        "sig_f": np.ascontiguousarray(
            np.asarray(sigma_filtered, dtype=np.float32).reshape(ROWS, 4 * Z)),
        "mu_p": np.ascontiguousarray(
            np.asarray(mu_pred, dtype=np.float32).reshape(ROWS, LAT)),
        "sig_p": np.ascontiguousarray(
            np.asarray(sigma_pred, dtype=np.float32).reshape(ROWS, 4 * Z)),
        "eps": np.ascontiguousarray(
            np.asarray(eps, dtype=np.float32).reshape(ROWS, LAT)),
    }
    in_maps = []
    for c in range(NCORES):
        sl = slice(c * DC, (c + 1) * DC)
        in_maps.append({
            **smalls,
            "tgt": np.ascontiguousarray(tgt[:, sl]),
            "wb": np.ascontiguousarray(wbf[:, sl]),
            "log_r": np.ascontiguousarray(lr[sl]),
        })
    return in_maps


def combine(results):
    sse = 0.0
    slr = 0.0
    for c in range(NCORES):
        v = results[c]["out"]
        sse += float(v[0]) + float(v[1])
        slr += float(v[2]) + float(v[3])
    klraw = float(results[0]["out"][4])
    n_tot = ROWS * D_OBS
    loss_integral = 0.5 * (
        n_tot * math.log(2.0 * math.pi) + 2.0 * ROWS * slr + sse
    ) / B
    loss_kl = 0.5 * (klraw - 2.0 * B * T * Z) / B
    return np.float32(loss_integral + loss_kl)


def kernel(mu_filtered, sigma_filtered, mu_pred, sigma_pred, target,
           W_dec, b_dec, log_R, eps):
    nc = _get_nc(1)
    in_maps = make_in_maps(mu_filtered, sigma_filtered, mu_pred, sigma_pred,
                           target, W_dec, b_dec, log_R, eps)
    res = run_bass_kernel_spmd(nc, in_maps, core_ids=list(range(NCORES)))
    return combine(res.results)



# revision 17
# speedup vs baseline: 1.1651x; 1.1651x over previous
"""Trainium2 Bass kernel for the DeepBayesianFilterBlockDiag loss.

Strategy (8-core SPMD, observation-axis sharded, TRANSPOSED layout):
  - The 152064-dim observation axis is split into 8 shards of 19008 columns,
    padded to 19072 = 149*128 per core.  The HOST pre-transposes each core's
    target shard to [149, 128, 256] (d-chunk, d-in-chunk, row) so the
    observation axis lands on SBUF/PSUM partitions; log_R likewise arrives
    as [128, 149].  W_dec||b_dec stays [65, 19072].
  - Per core:
      * phase 1: Xe = [mu_f + chol(sigma_f) @ eps, 1] and the KL terms
        (tiny per-(b,t,z) 2x2 algebra); -Xe^T [65,256] is the moving
        operand of the decode GEMM (float32r copy).
      * main loop over 75 PSUM banks (2 d-chunks each): PE injects the
        f32r target chunk-pair into the bank via one identity matmul
        (f32r moving streams at 1 col/cycle), then accumulates
        -Xe @ W' per 128-d chunk with W' slices as f32r stationaries
        (no bf16 conversion pass at all).  ACT squares each chunk
        IN-PLACE in PSUM with accum_out, yielding per-partition
        sums-of-squares directly into acc[:, chunk] — no separate
        subtract, no square tensor in SBUF, no colsum matmuls.
      * epilogue: sse = sum(acc * exp(-2 log_R)), plus sum(log_R) and the
        KL partial, emitted as a [3] vector.
  - Host combines the 8 partial vectors into the final scalar loss.
"""

import math

import numpy as np

import concourse.bass as bass
import concourse.mybir as mybir
import concourse.tile as tile
from concourse.bass_utils import run_bass_kernel_spmd
from concourse.masks import make_identity

F32 = mybir.dt.float32
F32R = mybir.dt.float32r
BF16 = mybir.dt.bfloat16
AF = mybir.ActivationFunctionType
OP = mybir.AluOpType

B, T, Z, DIM = 4, 64, 32, 2
ROWS = B * T          # 256
LAT = Z * DIM         # 64
LATP = LAT + 1        # 65 (ones row folds in b_dec)
D_OBS = 152064
NCORES = 8
DC = D_OBS // NCORES  # 19008 obs columns per core
NCC = 150             # 128-wide d-chunks per core (19200 = padded)
DCP = NCC * 128       # 19200
SEG = 16              # d-chunks per target DMA segment (8 banks)
BN_RES = (1, 2, 4)    # chunk residues (mod 5) handled by DVE bn_stats
ACT_RES = (0, 3)      # chunk residues handled by ACT square+accum
NGRP = NCC // 5       # 30 groups of 5 chunks
SCALAR_T_SEGS = (1, 4, 7)  # target segments issued on the scalar DGE ring
TP_BUFS = 3
DPS_BUFS = 6

MAX_DRAIN_WAITS = 1


def _split_multi_waits(nc, max_waits=1):
    """walrus' per-instruction sync encoding only fits one wait; move extra
    waits emitted by Tile onto NOPs inserted just before the instruction on
    the same engine (same semantics: engine blocks on all of them in order).
    """
    k = 0
    for f in nc.m.functions:
        for blk in f.blocks:
            il = blk.instructions
            i = 0
            while i < len(il):
                inst = il[i]
                si = inst.sync_info
                if si is not None and len(si.on_wait) > max_waits:
                    waits = list(si.on_wait)
                    inst.sync_info = mybir.SyncInfo(
                        on_wait=waits[-max_waits:], on_update=list(si.on_update)
                    )
                    extra = waits[:-max_waits]
                    for j in range(0, len(extra), max_waits):
                        nop = mybir.InstEventSemaphore(
                            name=f"{inst.name}-w{k}",
                            engine=inst.engine,
                            sync_info=mybir.SyncInfo(
                                on_wait=extra[j : j + max_waits], on_update=[]
                            ),
                        )
                        k += 1
                        il.insert(i, nop)
                        i += 1
                i += 1


def _comp4(t, mg, idx):
    # [128, 2, 128] tile -> [128, 32] view of 2x2-block component idx
    return t[:, mg, :].rearrange("p (z k) -> p z k", k=4)[:, :, idx]


def _comp2(t, mg, idx):
    return t[:, mg, :].rearrange("p (z k) -> p z k", k=2)[:, :, idx]


def build_nc(reps: int = 1, split_waits: bool = True, dup: int = 1):
    nc = bass.Bass("TRN2")
    tgt = nc.dram_tensor("tgt", [NCC, 128, ROWS], BF16, kind="ExternalInput")
    wb = nc.dram_tensor("wb", [LATP, DCP], BF16, kind="ExternalInput")
    lrt = nc.dram_tensor("log_r_t", [128, NCC], F32, kind="ExternalInput")
    muf = nc.dram_tensor("mu_f", [ROWS, LAT], F32, kind="ExternalInput")
    sgf = nc.dram_tensor("sig_f", [ROWS, 4 * Z], F32, kind="ExternalInput")
    mup = nc.dram_tensor("mu_p", [ROWS, LAT], F32, kind="ExternalInput")
    sgp = nc.dram_tensor("sig_p", [ROWS, 4 * Z], F32, kind="ExternalInput")
    eps = nc.dram_tensor("eps", [ROWS, LAT], F32, kind="ExternalInput")
    out = nc.dram_tensor("out", [6], F32, kind="ExternalOutput")

    with tile.TileContext(nc) as tc:
        with (
            tc.tile_pool(name="big", bufs=1) as big,
            tc.tile_pool(name="tp", bufs=TP_BUFS) as tpool,
            tc.tile_pool(name="small", bufs=1) as small,
            tc.tile_pool(name="pp2", bufs=2) as pp2,
            tc.tile_pool(name="dps", bufs=DPS_BUFS, space="PSUM") as dpsum,
            tc.tile_pool(name="smallps", bufs=1, space="PSUM") as smallps,
        ):
            if reps == 1:
                for _ in range(dup):
                    _body(nc, tc, big, tpool, small, pp2, dpsum, smallps,
                          tgt, wb, lrt, muf, sgf, mup, sgp, eps, out)
            else:
                with tc.For_i(0, reps, 1):
                    for _ in range(dup):
                        _body(nc, tc, big, tpool, small, pp2, dpsum, smallps,
                              tgt, wb, lrt, muf, sgf, mup, sgp, eps, out)
    if split_waits:
        # needed for the walrus/HW path; CoreSim wants the raw form
        _split_multi_waits(nc)
    return nc


def _body(nc, tc, big, tpool, small, pp2, dpsum, smallps,
          tgt, wb, lrt, muf, sgf, mup, sgp, eps, out):
    identf = small.tile([128, 128], F32)
    make_identity(nc, identf)
    ident = small.tile([128, 128], BF16)
    nc.vector.tensor_copy(ident, identf)
    ones = small.tile([128, 1], F32)
    nc.vector.memset(ones, 1.0)

    # ---- small inputs (all DMA on the sync ring; ACT/DVE streams stay pure) ----
    sigf_s = small.tile([128, 2, 4 * Z], F32)
    sigp_s = small.tile([128, 2, 4 * Z], F32)
    muf_s = small.tile([128, 2, LAT], F32)
    mup_s = small.tile([128, 2, LAT], F32)
    eps_s = small.tile([128, 2, LAT], F32)
    for mg in range(2):
        rs = slice(mg * 128, (mg + 1) * 128)
        nc.sync.dma_start(out=sigf_s[:, mg, :], in_=sgf[rs, :])
        nc.sync.dma_start(out=sigp_s[:, mg, :], in_=sgp[rs, :])
        nc.sync.dma_start(out=muf_s[:, mg, :], in_=muf[rs, :])
        nc.sync.dma_start(out=mup_s[:, mg, :], in_=mup[rs, :])
        nc.sync.dma_start(out=eps_s[:, mg, :], in_=eps[rs, :])
    lrt_s = pp2.tile([128, NCC], F32)
    nc.sync.dma_start(out=lrt_s, in_=lrt[:, :])

    # ---- phase 1: Xe (cholesky sample) + KL, per 128-row group ----
    lhsT = pp2.tile([LATP, 256], F32)
    nc.vector.memset(lhsT[LAT:LATP, :], -1.0)
    kl2 = pp2.tile([128, 2], F32)

    for mg in range(2):
        af = _comp4(sigf_s, mg, 0)
        bf = _comp4(sigf_s, mg, 1)
        cf = _comp4(sigf_s, mg, 2)
        df = _comp4(sigf_s, mg, 3)
        aq = _comp4(sigp_s, mg, 0)
        bq = _comp4(sigp_s, mg, 1)
        cq = _comp4(sigp_s, mg, 2)
        dq = _comp4(sigp_s, mg, 3)

        # cholesky: l11 = sqrt(a); l21 = c/l11; l22 = sqrt(d - l21^2)
        l11 = small.tile([128, Z], F32)
        nc.scalar.sqrt(l11, af)
        r11 = small.tile([128, Z], F32)
        nc.vector.reciprocal(r11, l11)
        l21 = small.tile([128, Z], F32)
        nc.vector.tensor_mul(l21, cf, r11)
        tmp0 = small.tile([128, Z], F32)
        nc.vector.tensor_mul(tmp0, l21, l21)
        nc.vector.tensor_sub(tmp0, df, tmp0)
        l22 = small.tile([128, Z], F32)
        nc.scalar.sqrt(l22, tmp0)

        e1 = _comp2(eps_s, mg, 0)
        e2 = _comp2(eps_s, mg, 1)
        m1 = _comp2(muf_s, mg, 0)
        m2 = _comp2(muf_s, mg, 1)

        xew = small.tile([128, LAT], F32)
        x1v = xew.rearrange("p (z k) -> p z k", k=2)[:, :, 0]
        x2v = xew.rearrange("p (z k) -> p z k", k=2)[:, :, 1]
        tA = small.tile([128, Z], F32)
        nc.vector.tensor_mul(tA, l11, e1)
        nc.vector.tensor_add(x1v, tA, m1)
        tB = small.tile([128, Z], F32)
        nc.vector.tensor_mul(tB, l21, e1)
        tC = small.tile([128, Z], F32)
        nc.vector.tensor_mul(tC, l22, e2)
        nc.vector.tensor_add(tB, tB, tC)
        nc.vector.tensor_add(x2v, tB, m2)

        tps = smallps.tile([LAT, 128], F32, tag="sps")
        nc.tensor.transpose(tps, xew, identf)
        nc.scalar.mul(lhsT[0:LAT, mg * 128 : (mg + 1) * 128], tps, -1.0)

        # KL pieces
        detq = small.tile([128, Z], F32)
        tD = small.tile([128, Z], F32)
        nc.vector.tensor_mul(detq, aq, dq)
        nc.vector.tensor_mul(tD, bq, cq)
        nc.vector.tensor_sub(detq, detq, tD)
        detp = small.tile([128, Z], F32)
        nc.vector.tensor_mul(detp, af, df)
        nc.vector.tensor_mul(tD, bf, cf)
        nc.vector.tensor_sub(detp, detp, tD)
        rdq = small.tile([128, Z], F32)
        nc.vector.reciprocal(rdq, detq)

        # trace numerator: dq*af - bq*bf - cq*cf + aq*df
        tn = small.tile([128, Z], F32)
        nc.vector.tensor_mul(tn, dq, af)
        nc.vector.tensor_mul(tD, aq, df)
        nc.vector.tensor_add(tn, tn, tD)
        nc.vector.tensor_mul(tD, bq, bf)
        nc.vector.tensor_sub(tn, tn, tD)
        nc.vector.tensor_mul(tD, cq, cf)
        nc.vector.tensor_sub(tn, tn, tD)

        # quad numerator: dq*d1^2 - (bq+cq)*d1*d2 + aq*d2^2
        p1 = _comp2(mup_s, mg, 0)
        p2 = _comp2(mup_s, mg, 1)
        d1 = small.tile([128, Z], F32)
        nc.vector.tensor_sub(d1, p1, m1)
        d2 = small.tile([128, Z], F32)
        nc.vector.tensor_sub(d2, p2, m2)
        qn = small.tile([128, Z], F32)
        nc.vector.tensor_mul(tD, d1, d1)
        nc.vector.tensor_mul(qn, dq, tD)
        nc.vector.tensor_mul(tD, d2, d2)
        nc.vector.tensor_mul(tD, aq, tD)
        nc.vector.tensor_add(qn, qn, tD)
        nc.vector.tensor_mul(tD, d1, d2)
        tE = small.tile([128, Z], F32)
        nc.vector.tensor_add(tE, bq, cq)
        nc.vector.tensor_mul(tD, tD, tE)
        nc.vector.tensor_sub(qn, qn, tD)

        klv = small.tile([128, Z], F32)
        nc.vector.tensor_add(klv, tn, qn)
        nc.vector.tensor_mul(klv, klv, rdq)
        # + ln(detq) - ln(detp)
        nc.scalar.activation(tD, detq, AF.Ln)
        nc.vector.tensor_add(klv, klv, tD)
        nc.scalar.activation(tD, detp, AF.Ln)
        nc.vector.tensor_sub(klv, klv, tD)
        nc.vector.reduce_sum(out=kl2[:, mg : mg + 1], in_=klv, axis=mybir.AxisListType.X)

    # moving operand of the decode GEMM: -Xe^T as bf16
    lhsT_r = pp2.tile([LATP, 256], BF16)
    nc.vector.tensor_copy(lhsT_r, lhsT)

    # w = exp(-2 log_R), already [128, NCC] on partitions
    w150 = pp2.tile([128, NCC], F32)
    nc.scalar.activation(w150, lrt_s, AF.Exp, scale=-2.0)

    # ---- W' (with b_dec row) resident in SBUF as f32r, on the scalar ring,
    # interleaved with the target segments on the same (sync) ring ----
    wb_s = big.tile([LATP, DCP], BF16)
    WBSEG = 3840
    wb_offs = list(range(0, DCP, WBSEG))

    # per-chunk sums of squares: ACT chunks accumulate into acc columns; DVE
    # chunks leave bn_stats moments to recover in the epilogue
    acc = pp2.tile([128, NCC], F32)
    nc.vector.memset(acc, 0.0)
    stats = pp2.tile([128, 3, NGRP, 6], F32)

    def issue_wb(upto):
        while wb_offs and wb_offs[0] < upto:
            woff = wb_offs.pop(0)
            ww = min(WBSEG, DCP - woff)
            nc.sync.dma_start(
                out=wb_s[:, woff : woff + ww],
                in_=wb[:, woff : woff + ww],
            )

    # ---- phase 2: main loop over target segments / psum banks ----
    issue_wb(2 * WBSEG)  # wb segs 0-1 up front
    for s0 in range(0, NCC, SEG):
        g = min(SEG, NCC - s0)
        t_s = tpool.tile([128, SEG, ROWS], BF16)
        nc.sync.dma_start(
            out=t_s[:, 0:g, :],
            in_=tgt[s0 : s0 + g, :, :].rearrange("g p r -> p g r"),
        )
        # keep the wb stream ~2 segments ahead of the mains consumers
        issue_wb((s0 + 2 * SEG) * 128)
        for b0 in range(0, g, 2):
            dps = dpsum.tile([128, 512], F32)
            nc.tensor.matmul(
                dps,
                lhsT=ident,
                rhs=t_s[:, b0 : b0 + 2, :].rearrange("p g r -> p (g r)"),
                start=True,
                stop=False,
            )
            for c in range(2):
                ch = s0 + b0 + c
                nc.tensor.matmul(
                    dps[:, c * ROWS : (c + 1) * ROWS],
                    lhsT=wb_s[:, ch * 128 : (ch + 1) * 128],
                    rhs=lhsT_r,
                    start=False,
                    stop=(c == 1),
                )
            for c in range(2):
                ch = s0 + b0 + c
                r = ch % 5
                pch = dps[:, c * ROWS : (c + 1) * ROWS]
                if r in ACT_RES:
                    nc.scalar.activation(
                        pch, pch, AF.Square, accum_out=acc[:, ch : ch + 1]
                    )
                else:
                    ri = BN_RES.index(r)
                    nc.vector.bn_stats(stats[:, ri, ch // 5, :], pch)

    # ---- phase 3: epilogue ----
    # combo columns: 0 = sse(act), 1..3 = sse(bn residues), 4 = sum(logR),
    #                5 = kl_raw
    combo = pp2.tile([128, 6], F32)
    nc.vector.memset(combo, 0.0)
    prod = pp2.tile([128, NCC], F32)
    nc.vector.tensor_mul(prod, acc, w150)
    nc.vector.reduce_sum(out=combo[:, 0:1], in_=prod, axis=mybir.AxisListType.X)
    wv5 = w150.rearrange("p (g k) -> p g k", k=5)
    tb1 = pp2.tile([128, NGRP], F32)
    tb2 = pp2.tile([128, NGRP], F32)
    for ri, r in enumerate(BN_RES):
        me = stats[:, ri, :, 1]
        m2e = stats[:, ri, :, 2]
        mo = stats[:, ri, :, 4]
        m2o = stats[:, ri, :, 5]
        nc.vector.tensor_mul(tb1, me, me)
        nc.vector.tensor_mul(tb2, mo, mo)
        nc.vector.tensor_add(tb1, tb1, tb2)
        nc.vector.tensor_add(tb2, m2e, m2o)
        # sq = 128*(me^2+mo^2) + (m2e+m2o)
        nc.vector.scalar_tensor_tensor(
            tb1, tb1, 128.0, tb2, op0=OP.mult, op1=OP.add
        )
        nc.vector.tensor_mul(tb1, tb1, wv5[:, :, r])
        nc.vector.reduce_sum(
            out=combo[:, 1 + ri : 2 + ri], in_=tb1, axis=mybir.AxisListType.X
        )
    nc.vector.reduce_sum(out=combo[:, 4:5], in_=lrt_s, axis=mybir.AxisListType.X)
    nc.vector.tensor_add(combo[:, 5:6], kl2[:, 0:1], kl2[:, 1:2])

    fps = smallps.tile([6, 1], F32, tag="sps")
    nc.tensor.matmul(fps, lhsT=combo, rhs=ones, start=True, stop=True)
    res = small.tile([6, 1], F32)
    nc.scalar.copy(res, fps)
    nc.sync.dma_start(out=out[:].rearrange("(p f) -> p f", f=1), in_=res)


_CACHED_NC = {}


def _get_nc(reps: int = 1):
    if reps not in _CACHED_NC:
        _CACHED_NC[reps] = build_nc(reps)
    return _CACHED_NC[reps]


def make_in_maps(mu_filtered, sigma_filtered, mu_pred, sigma_pred, target,
                 W_dec, b_dec, log_R, eps):
    tgt = np.asarray(target, dtype=np.float32).reshape(ROWS, D_OBS)
    wbf = np.concatenate(
        [np.asarray(W_dec, dtype=np.float32),
         np.asarray(b_dec, dtype=np.float32)[None, :]], axis=0
    )
    lr = np.asarray(log_R, dtype=np.float32)
    smalls = {
        "mu_f": np.ascontiguousarray(
            np.asarray(mu_filtered, dtype=np.float32).reshape(ROWS, LAT)),
        "sig_f": np.ascontiguousarray(
            np.asarray(sigma_filtered, dtype=np.float32).reshape(ROWS, 4 * Z)),
        "mu_p": np.ascontiguousarray(
            np.asarray(mu_pred, dtype=np.float32).reshape(ROWS, LAT)),
        "sig_p": np.ascontiguousarray(
            np.asarray(sigma_pred, dtype=np.float32).reshape(ROWS, 4 * Z)),
        "eps": np.ascontiguousarray(
            np.asarray(eps, dtype=np.float32).reshape(ROWS, LAT)),
    }
    import ml_dtypes

    bf16 = ml_dtypes.bfloat16
    in_maps = []
    for c in range(NCORES):
        sl = slice(c * DC, (c + 1) * DC)
        tgt_t = np.zeros((DCP, ROWS), dtype=bf16)
        tgt_t[:DC] = tgt[:, sl].T.astype(bf16)
        wbp = np.zeros((LATP, DCP), dtype=bf16)
        wbp[:, :DC] = wbf[:, sl].astype(bf16)
        lrp = np.zeros(DCP, dtype=np.float32)
        lrp[:DC] = lr[sl]
        in_maps.append({
            **smalls,
            "tgt": np.ascontiguousarray(tgt_t.reshape(NCC, 128, ROWS)),
            "wb": np.ascontiguousarray(wbp),
            "log_r_t": np.ascontiguousarray(lrp.reshape(NCC, 128).T),
        })
    return in_maps


def combine(results):
    sse = 0.0
    slr = 0.0
    for c in range(NCORES):
        v = results[c]["out"]
        sse += float(v[0]) + float(v[1]) + float(v[2]) + float(v[3])
        slr += float(v[4])
    klraw = float(results[0]["out"][5])
    n_tot = ROWS * D_OBS
    loss_integral = 0.5 * (
        n_tot * math.log(2.0 * math.pi) + 2.0 * ROWS * slr + sse
    ) / B
    loss_kl = 0.5 * (klraw - 2.0 * B * T * Z) / B
    return np.float32(loss_integral + loss_kl)


def kernel(mu_filtered, sigma_filtered, mu_pred, sigma_pred, target,
           W_dec, b_dec, log_R, eps):
    nc = _get_nc(1)
    in_maps = make_in_maps(mu_filtered, sigma_filtered, mu_pred, sigma_pred,
                           target, W_dec, b_dec, log_R, eps)
    res = run_bass_kernel_spmd(nc, in_maps, core_ids=list(range(NCORES)))
    return combine(res.results)
